# revision 28
# baseline (speedup 1.0000x reference)
"""Trainium2 Bass kernel for nn_InternalMAFE_59270548684863.

Key facts (hardcoded from the problem):
  - Output depends ONLY on branch 1 (p=7, n=288) of the reference; the
    n2=1008 branch feeds a dead projection and is never computed.
  - out = o1 @ proj_len_w.T + proj_len_b,  o1 = branch(x, 7, h1, w_k1, w_v1, ...)
  - Softmax normalizes over the batch axis, so we batch-shard (512 rows/core)
    and AllReduce the per-(slice, feature) exp-sums. Constant-shift softmax
    (exp(s*scale - 50)) avoids a cross-core max pass.
  - s = h1 @ (x_i w_k)^T is fused as W_hk = h1 @ w_k^T (one 288^3 product).
  - All matmuls run in bf16; PSUM accumulation stays fp32.

v4 schedule:
  - Contiguous fp32->bf16 casts; the feature de-interleave (stride 7) happens
    inside the PE transposes via strided stationary-operand views.
  - Per-slice pipeline: transposes(i+1) | logits(i)+exp(i) | vT(i)+z-mul(i),
    keeping PE dense.  Split AllReduce (slices 0-3 after slice 3, 4-6 after
    slice 6).  The plw pipeline has all-resident bf16 buffers and its casts
    are emitted mid-FIFO so nothing stalls.
  - Scan state is bf16 and doubles as the projection lhsT; normalization is
    folded into the scan via scalar_tensor_tensor.  The projection
    accumulates per-slice into 8 PSUM banks while the scan runs.
"""

import math

import numpy as np

import concourse.bacc as bacc
import concourse.masks as masks
import concourse.mybir as mybir
import concourse.tile as tile
from concourse.bass_utils import run_bass_kernel_spmd

N_CORES = 8
B = 4096
BL = B // N_CORES  # 512 rows per core
INP = 2016
P1 = 7
N1 = 288
SEQ = 1024
SCALE = 1.0 / math.sqrt(N1)
SHIFT = -50.0
F32 = mybir.dt.float32
BF16 = mybir.dt.bfloat16
CH = [(0, 128), (128, 128), (256, 32)]
AF = mybir.ActivationFunctionType
ALU = mybir.AluOpType


def build():
    nc = bacc.Bacc(
        "TRN2", target_bir_lowering=False, debug=False, num_devices=N_CORES
    )
    x = nc.dram_tensor("x", [BL, INP], F32, kind="ExternalInput").ap()
    wk = nc.dram_tensor("w_k1", [N1, N1], F32, kind="ExternalInput").ap()
    wv = nc.dram_tensor("w_v1", [N1, N1], F32, kind="ExternalInput").ap()
    h1 = nc.dram_tensor("h1", [N1, N1], F32, kind="ExternalInput").ap()
    a1 = nc.dram_tensor("alpha1", [1], F32, kind="ExternalInput").ap()
    a2 = nc.dram_tensor("alpha2", [1], F32, kind="ExternalInput").ap()
    b1 = nc.dram_tensor("beta1", [1], F32, kind="ExternalInput").ap()
    b2 = nc.dram_tensor("beta2", [1], F32, kind="ExternalInput").ap()
    plw = nc.dram_tensor("proj_len_w", [SEQ, INP], F32, kind="ExternalInput").ap()
    plb = nc.dram_tensor("proj_len_b", [SEQ], F32, kind="ExternalInput").ap()
    out = nc.dram_tensor("out", [BL, SEQ], F32, kind="ExternalOutput").ap()

    def deint(ap_2d, i, j0, cnt):
        # strided view of a [128, INP] tile: columns (j0+jj)*7 + i
        v = ap_2d.rearrange("p (j i) -> p j i", i=P1)
        return v[:, j0 : j0 + cnt, i : i + 1].rearrange("p j i -> p (j i)")

    with tile.TileContext(nc) as tc:
        with (
            tc.tile_pool(name="const", bufs=1) as cpool,
            tc.tile_pool(name="zz", bufs=1) as zpool,
            tc.tile_pool(name="rk", bufs=1) as rkpool,
            tc.tile_pool(name="dram", bufs=1, space="DRAM") as dpool,
        ):
            # ---------------- constants ----------------
            ident = cpool.tile([128, 128], BF16, tag="ident", name="ident")
            masks.make_identity(nc, ident[:])
            ones_bf = cpool.tile([1, 128], BF16, tag="ones_bf", name="ones_bf")
            nc.vector.memset(ones_bf[:], 1.0)
            onesf = cpool.tile([1, 128], F32, tag="onesf", name="onesf")
            nc.vector.memset(onesf[:], 1.0)

            scal = cpool.tile([1, 4], F32, tag="scal", name="scal")
            for idx, ap in enumerate((a1, a2, b1, b2)):
                nc.sync.dma_start(scal[0:1, idx : idx + 1], ap[:])

            plb_f = cpool.tile([1, SEQ], F32, tag="plb_f", name="plb_f")
            nc.sync.dma_start(plb_f[:], plb[:])
            plb_sb = cpool.tile([1, SEQ], BF16, tag="plb", name="plb")
            nc.vector.tensor_copy(plb_sb[:], plb_f[:])

            densb = cpool.tile([128, 24], F32, tag="densb", name="densb")
            nc.vector.memset(densb[:], 0.0)
            shiftc = cpool.tile([128, 1], F32, tag="shiftc", name="shiftc")
            nc.vector.memset(shiftc[:], SHIFT)
            den_all = cpool.tile([128, 24], F32, tag="den_all", name="den_all")
            recip = cpool.tile([128, 24], F32, tag="recip", name="recip")

            cc_in = [
                dpool.tile([128, 12], F32, tag=f"cc_in{h}", name=f"cc_in{h}")
                for h in range(2)
            ]
            cc_out = [
                dpool.tile(
                    [128, 12], F32, addr_space="Shared",
                    tag=f"cc_out{h}", name=f"cc_out{h}",
                )
                for h in range(2)
            ]

            # scan state, bf16 (doubles as projection lhsT):
            # c0|c1 packed [128, 1024]; c2 packed 3-slices-per-tile [96, 512]
            zA = [
                zpool.tile([128, 2 * BL], BF16, tag=f"zA{i}", name=f"zA{i}")
                for i in range(P1)
            ]
            zC = [
                zpool.tile([32, BL], BF16, tag=f"zC{i}", name=f"zC{i}")
                for i in range(P1)
            ]
            # de-interleaved plw^T tiles
            rk = [
                [
                    rkpool.tile([128, SEQ], BF16, tag=f"rk{i}_{c}", name=f"rk{i}_{c}")
                    for c in range(2)
                ]
                for i in range(P1)
            ]
            rk2 = [
                rkpool.tile([32, SEQ], BF16, tag=f"rkc2_{i}", name=f"rkc2_{i}")
                for i in range(P1)
            ]

            # ---------------- phase A ----------------
            with (
                tc.tile_pool(name="wst", bufs=3) as wstage,
                tc.tile_pool(name="wbf", bufs=1) as wpool,
                tc.tile_pool(name="stg", bufs=2) as stage,
                tc.tile_pool(name="xb", bufs=4) as xbpool,
                tc.tile_pool(name="pb16", bufs=8) as pwbpool,
                tc.tile_pool(name="xiT", bufs=3) as xtpool,
                tc.tile_pool(name="ee", bufs=3) as epool,
                tc.tile_pool(name="psT", bufs=3, space="PSUM") as psT,
                tc.tile_pool(name="psS", bufs=2, space="PSUM") as psS,
                tc.tile_pool(name="psV", bufs=2, space="PSUM") as psV,
                tc.tile_pool(name="psT2", bufs=1, space="PSUM") as psT2,
            ):
                EA = [
                    epool.tile([128, 2 * BL], BF16, tag="EA", name=f"EA{i}")
                    for i in range(P1)
                ]
                EC = [
                    epool.tile([32, BL], BF16, tag="EC", name=f"EC{i}")
                    for i in range(P1)
                ]

                # PE warm-up: dependency-free transposes so the HAM clock
                # gate opens during the initial DMA window (results unused)
                for w in range(90):
                    psj = psS.tile([128, 512], BF16, tag="ps_st", name="ps_jk")
                    nc.tensor.transpose(psj[:, 0:128], ident[:], ident[:])

                # gate scalars -> all 128 partitions via PE
                pbc = psS.tile([128, 512], F32, tag="ps_st", name="ps_bc")
                nc.tensor.matmul(pbc[:, 0:4], onesf[:], scal[:], start=True, stop=True)
                bcast = cpool.tile([128, 4], F32, tag="bcast", name="bcast")
                nc.vector.tensor_copy(bcast[:], pbc[:, 0:4])

                def load_w(src, nm):
                    tiles = []
                    for t, (m0, mc) in enumerate(CH):
                        wtf = wstage.tile([mc, N1], F32, tag="wtmp", name="wtmp")
                        nc.sync.dma_start(wtf[:], src[m0 : m0 + mc, :])
                        wt = wpool.tile(
                            [mc, N1], BF16, tag=f"{nm}{t}", name=f"{nm}{t}"
                        )
                        nc.vector.tensor_copy(wt[:], wtf[:])
                        tiles.append(wt)
                    return tiles

                wk_b = load_w(wk, "wkb")
                h1_b = load_w(h1, "h1b")
                wv_b = load_w(wv, "wvb")

                # x shard: fp32 load -> contiguous bf16 cast
                xbt = []
                for bt in range(4):
                    xt = stage.tile([128, INP], F32, tag="stg", name="stg")
                    nc.sync.dma_start(xt[:], x[bt * 128 : (bt + 1) * 128, :])
                    xb = xbpool.tile([128, INP], BF16, tag="xnb", name="xnb")
                    nc.vector.tensor_copy(xb[:], xt[:])
                    xbt.append(xb)

                # plw DMAs (queue behind x; casts are emitted later mid-FIFO)
                pwstage = []
                for st in range(8):
                    pwt = stage.tile([128, INP], F32, tag="stg", name="stg")
                    nc.sync.dma_start(pwt[:], plw[st * 128 : (st + 1) * 128, :])
                    pwstage.append(pwt)
                pwb = [None] * 8

                def emit_plw_cast(st):
                    pb = pwbpool.tile([128, INP], BF16, tag="plwb", name="plwb")
                    nc.vector.tensor_copy(pb[:], pwstage[st][:])
                    pwb[st] = pb

                # h1T[l, j] = h1[j, l] and wkT[l, m] = wk[m, l]  (bf16)
                h1T, wkT = [], []
                for lt, (l0, lc) in enumerate(CH):
                    ps = psT.tile([128, 512], BF16, tag="tp", name="tp")
                    for jt, (j0, jc) in enumerate(CH):
                        nc.tensor.transpose(
                            ps[0:lc, j0 : j0 + jc],
                            h1_b[jt][0:jc, l0 : l0 + lc],
                            ident[0:jc, 0:jc],
                        )
                    hT = wpool.tile([lc, N1], BF16, tag=f"h1T{lt}", name=f"h1T{lt}")
                    nc.vector.tensor_copy(hT[:], ps[0:lc, 0:N1])
                    h1T.append(hT)
                    ps2 = psT.tile([128, 512], BF16, tag="tp", name="tp")
                    for mt, (m0, mc) in enumerate(CH):
                        nc.tensor.transpose(
                            ps2[0:lc, m0 : m0 + mc],
                            wk_b[mt][0:mc, l0 : l0 + lc],
                            ident[0:mc, 0:mc],
                        )
                    wTl = wpool.tile([lc, N1], BF16, tag=f"wkT{lt}", name=f"wkT{lt}")
                    nc.vector.tensor_copy(wTl[:], ps2[0:lc, 0:N1])
                    wkT.append(wTl)

                # W_hkT[m, j] = sum_l wk[m,l] h1[j,l]
                whkT = []
                for mt, (m0, mc) in enumerate(CH):
                    pw = psS.tile([128, 512], F32, tag="ps_st", name="ps_whk")
                    for lt, (l0, lc) in enumerate(CH):
                        nc.tensor.matmul(
                            pw[0:mc, 0:N1],
                            wkT[lt][:, m0 : m0 + mc],
                            h1T[lt][:],
                            start=(lt == 0),
                            stop=(lt == 2),
                        )
                    wTt = wpool.tile(
                        [mc, N1], BF16, tag=f"whkT{mt}", name=f"whkT{mt}"
                    )
                    nc.vector.tensor_copy(wTt[:], pw[0:mc, 0:N1])
                    whkT.append(wTt)

                # per-slice pipeline ---------------------------------------
                xiT = [[None] * 3 for _ in range(P1)]

                def emit_transposes(i):
                    for c in range(2):
                        j0 = c * 128
                        ps = psT.tile([128, 512], BF16, tag="tp", name="tp")
                        for bt in range(4):
                            nc.tensor.transpose(
                                ps[:, bt * 128 : (bt + 1) * 128],
                                deint(xbt[bt][:], i, j0, 128),
                                ident[:],
                            )
                        xi = xtpool.tile(
                            [128, BL], BF16, tag=f"xc{c}", name=f"xiT{i}_{c}"
                        )
                        nc.vector.tensor_copy(xi[:], ps[:])
                        xiT[i][c] = xi
                    ps = psT.tile([128, 512], BF16, tag="tp", name="tp")
                    for bt in range(4):
                        nc.tensor.transpose(
                            ps[0:32, bt * 128 : (bt + 1) * 128],
                            deint(xbt[bt][:], i, 256, 32),
                            ident[:],
                        )
                    xi = xtpool.tile([32, BL], BF16, tag="xc2", name=f"xiT{i}_2")
                    nc.vector.tensor_copy(xi[:], ps[0:32, :])
                    xiT[i][2] = xi

                def emit_logits(i):
                    for jt, (j0, jc) in enumerate(CH):
                        pst = psS.tile([128, 512], F32, tag="ps_st", name="ps_st")
                        for lt, (l0, lc) in enumerate(CH):
                            nc.tensor.matmul(
                                pst[0:jc, :],
                                whkT[lt][:, j0 : j0 + jc],
                                xiT[i][lt][:],
                                start=(lt == 0),
                                stop=(lt == 2),
                            )
                        col = i * 3 + jt
                        if jt < 2:
                            eout = EA[i][:, jt * BL : (jt + 1) * BL]
                        else:
                            eout = EC[i][:]
                        nc.scalar.activation(
                            eout,
                            pst[0:jc, :],
                            AF.Exp,
                            bias=shiftc[0:jc, 0:1],
                            scale=SCALE,
                            accum_out=densb[0:jc, col : col + 1],
                        )

                def emit_vt(i):
                    for ntc, (n0, ncnt) in enumerate(CH):
                        pv = psV.tile([128, 512], F32, tag="ps_vt", name="ps_vt")
                        for mt, (m0, mc) in enumerate(CH):
                            nc.tensor.matmul(
                                pv[0:ncnt, :],
                                wv_b[mt][:, n0 : n0 + ncnt],
                                xiT[i][mt][:],
                                start=(mt == 0),
                                stop=(mt == 2),
                            )
                        if ntc < 2:
                            zv = zA[i][:, ntc * BL : (ntc + 1) * BL]
                            ev = EA[i][:, ntc * BL : (ntc + 1) * BL]
                        else:
                            zv = zC[i][:]
                            ev = EC[i][:]
                        nc.vector.tensor_mul(zv, pv[0:ncnt, :], ev)

                emit_transposes(0)
                for i in range(P1):
                    if i + 1 < P1:
                        emit_transposes(i + 1)
                    emit_logits(i)
                    if i >= 2 and i <= 5:  # plw casts mid-FIFO (DMAs done by now)
                        emit_plw_cast(2 * (i - 2))
                        emit_plw_cast(2 * (i - 2) + 1)
                    if i == 3:
                        nc.gpsimd.dma_start(cc_in[0][:], densb[:, 0:12])
                        nc.gpsimd.collective_compute(
                            "AllReduce",
                            ALU.add,
                            replica_groups=[list(range(N_CORES))],
                            ins=[cc_in[0][:]],
                            outs=[cc_out[0][:]],
                        )
                nc.gpsimd.dma_start(cc_in[1][:], densb[:, 12:24])
                nc.gpsimd.collective_compute(
                    "AllReduce",
                    ALU.add,
                    replica_groups=[list(range(N_CORES))],
                    ins=[cc_in[1][:]],
                    outs=[cc_out[1][:]],
                )

                # vT + z-muls fill the AllReduce window
                for i in range(P1):
                    emit_vt(i)

                # plw de-interleave (fills the AllReduce window), per seq-half
                for sh in range(2):
                    for i in range(P1):
                        for c in range(2):
                            j0 = c * 128
                            ps = psT2.tile([128, 512], BF16, tag="tp2", name="tp2")
                            for st in range(4):
                                nc.tensor.transpose(
                                    ps[:, st * 128 : (st + 1) * 128],
                                    deint(pwb[sh * 4 + st][:], i, j0, 128),
                                    ident[:],
                                )
                            dst = rk[i][c][:, sh * 512 : (sh + 1) * 512]
                            if (i + c) % 2 == 0:
                                nc.vector.tensor_copy(dst, ps[:])
                            else:
                                nc.scalar.copy(dst, ps[:])
                        ps = psT2.tile([128, 512], BF16, tag="tp2", name="tp2")
                        for st in range(4):
                            nc.tensor.transpose(
                                ps[0:32, st * 128 : (st + 1) * 128],
                                deint(pwb[sh * 4 + st][:], i, 256, 32),
                                ident[:],
                            )
                        nc.vector.tensor_copy(
                            rk2[i][:, sh * 512 : (sh + 1) * 512],
                            ps[0:32, :],
                        )

            # ---------------- AR consume ----------------
            nc.gpsimd.dma_start(den_all[:, 0:12], cc_out[0][:])
            nc.vector.reciprocal(recip[:, 0:12], den_all[:, 0:12])
            nc.gpsimd.dma_start(den_all[:, 12:24], cc_out[1][:])
            nc.vector.reciprocal(recip[:, 12:24], den_all[:, 12:24])
            den_bf = cpool.tile([128, 24], BF16, tag="den_bf", name="den_bf")
            nc.vector.tensor_copy(den_bf[:], den_all[:])

            # ---------------- scan + projection ----------------
            with (
                tc.tile_pool(name="tmp", bufs=1) as tmppool,
                tc.tile_pool(name="osb", bufs=2) as outpool,
                tc.tile_pool(name="psP", bufs=1, space="PSUM") as psP,
            ):
                ttA = tmppool.tile([128, 2 * BL], BF16, tag="ttA", name="ttA")
                tsA = tmppool.tile([128, 2 * BL], BF16, tag="tsA", name="tsA")
                gA = tmppool.tile([128, 2 * BL], BF16, tag="gA", name="gA")
                tt2 = tmppool.tile([32, BL], BF16, tag="tt2", name="tt2")
                ts2 = tmppool.tile([32, BL], BF16, tag="ts2", name="ts2")
                g2 = tmppool.tile([32, BL], BF16, tag="g2", name="g2")

                # re-warm the PE clock right before the projection: junk
                # matmuls anchored on the AllReduce result (overwritten by
                # the start=True proj_len_b pre-load below)
                wps = psP.tile([128, 512], F32, tag="pj00", name="pj_warm")
                for w in range(16):
                    nc.tensor.matmul(
                        wps[0:24, :], den_bf[:], rk[0][0][:, 0:512],
                        start=True, stop=True,
                    )

                # pre-load proj_len_b into the 8 projection PSUM banks
                pps = {}
                for half in range(2):
                    for bc in range(4):
                        pp = psP.tile(
                            [128, 512], F32, tag=f"pj{half}{bc}", name=f"pj{half}{bc}"
                        )
                        nc.tensor.matmul(
                            pp[:],
                            ones_bf[:],
                            plb_sb[0:1, half * 512 : (half + 1) * 512],
                            start=True,
                            stop=False,
                        )
                        pps[(half, bc)] = pp

                def proj_tile(lsrc, off, rsrc, last):
                    for half in range(2):
                        for bc in range(4):
                            nc.tensor.matmul(
                                pps[(half, bc)][:],
                                lsrc[:, off + bc * 128 : off + (bc + 1) * 128],
                                rsrc[:, half * 512 : (half + 1) * 512],
                                start=False,
                                stop=(last and half == 1 and bc == 3),
                            )

                for i in range(P1):
                    cols = [i * 3, i * 3 + 1, i * 3 + 2]
                    zc = zC[i][:]
                    if i == 0:
                        # y_0 = z_0 * recip
                        for c in range(2):
                            nc.scalar.mul(
                                zA[0][:, c * BL : (c + 1) * BL],
                                zA[0][:, c * BL : (c + 1) * BL],
                                mul=recip[0:128, cols[c] : cols[c] + 1],
                            )
                        nc.scalar.mul(
                            zc, zc, mul=recip[0:32, cols[2] : cols[2] + 1]
                        )
                    else:
                        # g = tanh(a1*y + b1) * sigmoid(a2*y + b2)
                        nc.scalar.activation(
                            ttA[:], zA[i - 1][:], AF.Tanh,
                            bias=bcast[0:128, 2:3], scale=bcast[0:128, 0:1],
                        )
                        nc.scalar.activation(
                            tsA[:], zA[i - 1][:], AF.Sigmoid,
                            bias=bcast[0:128, 3:4], scale=bcast[0:128, 1:2],
                        )
                        nc.scalar.activation(
                            tt2[:], zC[i - 1][:], AF.Tanh,
                            bias=bcast[0:32, 2:3], scale=bcast[0:32, 0:1],
                        )
                        nc.scalar.activation(
                            ts2[:], zC[i - 1][:], AF.Sigmoid,
                            bias=bcast[0:32, 3:4], scale=bcast[0:32, 1:2],
                        )
                        nc.vector.tensor_mul(gA[:], ttA[:], tsA[:])
                        nc.gpsimd.tensor_mul(g2[:], tt2[:], ts2[:])
                        # y_i = z_i * recip + g   (normalize folded in)
                        for c in range(2):
                            zv = zA[i][:, c * BL : (c + 1) * BL]
                            nc.vector.scalar_tensor_tensor(
                                zv, zv,
                                recip[0:128, cols[c] : cols[c] + 1],
                                gA[:, c * BL : (c + 1) * BL],
                                op0=ALU.mult, op1=ALU.add,
                            )
                        nc.vector.scalar_tensor_tensor(
                            zc, zc,
                            recip[0:32, cols[2] : cols[2] + 1],
                            g2[:],
                            op0=ALU.mult, op1=ALU.add,
                        )
                    proj_tile(zA[i], 0, rk[i][0], False)
                    proj_tile(zA[i], BL, rk[i][1], False)
                    proj_tile(zC[i], 0, rk2[i], last=(i == P1 - 1))

                for half in range(2):
                    for bc in range(4):
                        ob = outpool.tile([128, 512], F32, tag="osb", name="osb")
                        nc.scalar.copy(ob[:], pps[(half, bc)][:])
                        nc.sync.dma_start(
                            out[bc * 128 : (bc + 1) * 128, half * 512 : (half + 1) * 512],
                            ob[:],
                        )

    nc.compile()
    return nc


_NC = None


def _get_nc():
    global _NC
    if _NC is None:
        _NC = build()
    return _NC


def run(inputs, trace=False):
    nc = _get_nc()
    rep_keys = [
        "w_k1",
        "w_v1",
        "h1",
        "alpha1",
        "alpha2",
        "beta1",
        "beta2",
        "proj_len_w",
        "proj_len_b",
    ]
    x = np.ascontiguousarray(inputs["x"], dtype=np.float32)
    rep = {k: np.ascontiguousarray(inputs[k], dtype=np.float32) for k in rep_keys}
    in_maps = [
        {"x": x[c * BL : (c + 1) * BL], **rep} for c in range(N_CORES)
    ]
    res = run_bass_kernel_spmd(
        nc, in_maps, core_ids=list(range(N_CORES)), trace=trace
    )
    full = np.concatenate([res.results[c]["out"] for c in range(N_CORES)], axis=0)
    return full, res


def kernel(**inputs):
    full, _ = run(inputs, trace=False)
    return full


# revision 29
# speedup vs baseline: 1.1272x; 1.1272x over previous
"""Trainium2 Bass kernel for nn_InternalMAFE_59270548684863.

Key facts (hardcoded from the problem):
  - Output depends ONLY on branch 1 (p=7, n=288) of the reference; the
    n2=1008 branch feeds a dead projection and is never computed.
  - out = o1 @ proj_len_w.T + proj_len_b,  o1 = branch(x, 7, h1, w_k1, w_v1, ...)
  - Softmax normalizes over the batch axis, so we batch-shard (512 rows/core)
    and AllReduce the per-(slice, feature) exp-sums. Constant-shift softmax
    (exp(s*scale - 50)) avoids a cross-core max pass.
  - s = h1 @ (x_i w_k)^T is fused as W_hk = h1 @ w_k^T (one 288^3 product).
  - All matmuls run in bf16; PSUM accumulation stays fp32.

v4 schedule:
  - Contiguous fp32->bf16 casts; the feature de-interleave (stride 7) happens
    inside the PE transposes via strided stationary-operand views.
  - Per-slice pipeline: transposes(i+1) | logits(i)+exp(i) | vT(i)+z-mul(i),
    keeping PE dense.  Split AllReduce (slices 0-3 after slice 3, 4-6 after
    slice 6).  The plw pipeline has all-resident bf16 buffers and its casts
    are emitted mid-FIFO so nothing stalls.
  - Scan state is bf16 and doubles as the projection lhsT; normalization is
    folded into the scan via scalar_tensor_tensor.  The projection
    accumulates per-slice into 8 PSUM banks while the scan runs.
"""

import math

import numpy as np

import concourse.bacc as bacc
import concourse.masks as masks
import concourse.mybir as mybir
import concourse.tile as tile
from concourse.bass_utils import run_bass_kernel_spmd

N_CORES = 8
B = 4096
BL = B // N_CORES  # 512 rows per core
INP = 2016
P1 = 7
N1 = 288
SEQ = 1024
SCALE = 1.0 / math.sqrt(N1)
SHIFT = -50.0
F32 = mybir.dt.float32
BF16 = mybir.dt.bfloat16
CH = [(0, 128), (128, 128), (256, 32)]
AF = mybir.ActivationFunctionType
ALU = mybir.AluOpType


def build():
    nc = bacc.Bacc(
        "TRN2", target_bir_lowering=False, debug=False, num_devices=N_CORES
    )
    x = nc.dram_tensor("x", [BL, INP], F32, kind="ExternalInput").ap()
    wk = nc.dram_tensor("w_k1", [N1, N1], F32, kind="ExternalInput").ap()
    wv = nc.dram_tensor("w_v1", [N1, N1], F32, kind="ExternalInput").ap()
    h1 = nc.dram_tensor("h1", [N1, N1], F32, kind="ExternalInput").ap()
    a1 = nc.dram_tensor("alpha1", [1], F32, kind="ExternalInput").ap()
    a2 = nc.dram_tensor("alpha2", [1], F32, kind="ExternalInput").ap()
    b1 = nc.dram_tensor("beta1", [1], F32, kind="ExternalInput").ap()
    b2 = nc.dram_tensor("beta2", [1], F32, kind="ExternalInput").ap()
    plw = nc.dram_tensor("proj_len_w", [SEQ, INP], F32, kind="ExternalInput").ap()
    plb = nc.dram_tensor("proj_len_b", [SEQ], F32, kind="ExternalInput").ap()
    out = nc.dram_tensor("out", [BL, SEQ], F32, kind="ExternalOutput").ap()

    def deint(ap_2d, i, j0, cnt):
        # strided view of a [128, INP] tile: columns (j0+jj)*7 + i
        v = ap_2d.rearrange("p (j i) -> p j i", i=P1)
        return v[:, j0 : j0 + cnt, i : i + 1].rearrange("p j i -> p (j i)")

    with tile.TileContext(nc) as tc:
        with (
            tc.tile_pool(name="const", bufs=1) as cpool,
            tc.tile_pool(name="zz", bufs=1) as zpool,
            tc.tile_pool(name="rk", bufs=1) as rkpool,
            tc.tile_pool(name="dram", bufs=1, space="DRAM") as dpool,
        ):
            # ---------------- constants ----------------
            ident = cpool.tile([128, 128], BF16, tag="ident", name="ident")
            masks.make_identity(nc, ident[:])
            ones_bf = cpool.tile([1, 128], BF16, tag="ones_bf", name="ones_bf")
            nc.vector.memset(ones_bf[:], 1.0)
            onesf = cpool.tile([1, 128], F32, tag="onesf", name="onesf")
            nc.vector.memset(onesf[:], 1.0)

            scal = cpool.tile([1, 4], F32, tag="scal", name="scal")
            for idx, ap in enumerate((a1, a2, b1, b2)):
                nc.sync.dma_start(scal[0:1, idx : idx + 1], ap[:])

            plb_f = cpool.tile([1, SEQ], F32, tag="plb_f", name="plb_f")
            nc.sync.dma_start(plb_f[:], plb[:])
            plb_sb = cpool.tile([1, SEQ], BF16, tag="plb", name="plb")
            nc.vector.tensor_copy(plb_sb[:], plb_f[:])

            densb = cpool.tile([128, 24], F32, tag="densb", name="densb")
            nc.vector.memset(densb[:], 0.0)
            shiftc = cpool.tile([128, 1], F32, tag="shiftc", name="shiftc")
            nc.vector.memset(shiftc[:], SHIFT)
            den_all = cpool.tile([128, 24], F32, tag="den_all", name="den_all")
            recip = cpool.tile([128, 24], F32, tag="recip", name="recip")

            cc_in = [
                dpool.tile([128, 12], F32, tag=f"cc_in{h}", name=f"cc_in{h}")
                for h in range(2)
            ]
            cc_out = [
                dpool.tile(
                    [128, 12], F32, addr_space="Shared",
                    tag=f"cc_out{h}", name=f"cc_out{h}",
                )
                for h in range(2)
            ]

            # scan state, bf16 (doubles as projection lhsT):
            # c0|c1 packed [128, 1024]; c2 packed 3-slices-per-tile [96, 512]
            zA = [
                zpool.tile([128, 2 * BL], BF16, tag=f"zA{i}", name=f"zA{i}")
                for i in range(P1)
            ]
            zC = [
                zpool.tile([32, BL], BF16, tag=f"zC{i}", name=f"zC{i}")
                for i in range(P1)
            ]
            # de-interleaved plw^T tiles
            rk = [
                [
                    rkpool.tile([128, SEQ], BF16, tag=f"rk{i}_{c}", name=f"rk{i}_{c}")
                    for c in range(2)
                ]
                for i in range(P1)
            ]
            rk2 = [
                rkpool.tile([32, SEQ], BF16, tag=f"rkc2_{i}", name=f"rkc2_{i}")
                for i in range(P1)
            ]

            # ---------------- phase A ----------------
            with (
                tc.tile_pool(name="wst", bufs=3) as wstage,
                tc.tile_pool(name="wbf", bufs=1) as wpool,
                tc.tile_pool(name="stg", bufs=2) as stage,
                tc.tile_pool(name="xb", bufs=4) as xbpool,
                tc.tile_pool(name="pb16", bufs=8) as pwbpool,
                tc.tile_pool(name="xiT", bufs=3) as xtpool,
                tc.tile_pool(name="ee", bufs=3) as epool,
                tc.tile_pool(name="psT", bufs=2, space="PSUM") as psT,
                tc.tile_pool(name="psS", bufs=2, space="PSUM") as psS,
                tc.tile_pool(name="psV", bufs=2, space="PSUM") as psV,
                tc.tile_pool(name="psT2", bufs=2, space="PSUM") as psT2,
            ):
                EA = [
                    epool.tile([128, 2 * BL], BF16, tag="EA", name=f"EA{i}")
                    for i in range(P1)
                ]
                EC = [
                    epool.tile([32, BL], BF16, tag="EC", name=f"EC{i}")
                    for i in range(P1)
                ]

                # PE warm-up: dependency-free transposes so the HAM clock
                # gate opens during the initial DMA window (results unused)
                for w in range(90):
                    psj = psS.tile([128, 512], BF16, tag="ps_st", name="ps_jk")
                    nc.tensor.transpose(psj[:, 0:128], ident[:], ident[:])

                # gate scalars -> all 128 partitions via PE
                pbc = psS.tile([128, 512], F32, tag="ps_st", name="ps_bc")
                nc.tensor.matmul(pbc[:, 0:4], onesf[:], scal[:], start=True, stop=True)
                bcast = cpool.tile([128, 4], F32, tag="bcast", name="bcast")
                nc.vector.tensor_copy(bcast[:], pbc[:, 0:4])

                def load_w(src, nm):
                    tiles = []
                    for t, (m0, mc) in enumerate(CH):
                        wtf = wstage.tile([mc, N1], F32, tag="wtmp", name="wtmp")
                        nc.sync.dma_start(wtf[:], src[m0 : m0 + mc, :])
                        wt = wpool.tile(
                            [mc, N1], BF16, tag=f"{nm}{t}", name=f"{nm}{t}"
                        )
                        nc.vector.tensor_copy(wt[:], wtf[:])
                        tiles.append(wt)
                    return tiles

                wk_b = load_w(wk, "wkb")
                h1_b = load_w(h1, "h1b")
                wv_b = load_w(wv, "wvb")

                # x shard: fp32 load -> contiguous bf16 cast
                xbt = []
                for bt in range(4):
                    xt = stage.tile([128, INP], F32, tag="stg", name="stg")
                    nc.sync.dma_start(xt[:], x[bt * 128 : (bt + 1) * 128, :])
                    xb = xbpool.tile([128, INP], BF16, tag="xnb", name="xnb")
                    nc.vector.tensor_copy(xb[:], xt[:])
                    xbt.append(xb)

                # plw DMAs (queue behind x; casts are emitted later mid-FIFO)
                pwstage = []
                for st in range(8):
                    pwt = stage.tile([128, INP], F32, tag="stg", name="stg")
                    nc.sync.dma_start(pwt[:], plw[st * 128 : (st + 1) * 128, :])
                    pwstage.append(pwt)
                pwb = [None] * 8

                def emit_plw_cast(st):
                    pb = pwbpool.tile([128, INP], BF16, tag="plwb", name="plwb")
                    nc.vector.tensor_copy(pb[:], pwstage[st][:])
                    pwb[st] = pb

                # h1T[l, j] = h1[j, l] and wkT[l, m] = wk[m, l]  (bf16)
                h1T, wkT = [], []
                for lt, (l0, lc) in enumerate(CH):
                    ps = psT.tile([128, 512], BF16, tag="tp", name="tp")
                    for jt, (j0, jc) in enumerate(CH):
                        nc.tensor.transpose(
                            ps[0:lc, j0 : j0 + jc],
                            h1_b[jt][0:jc, l0 : l0 + lc],
                            ident[0:jc, 0:jc],
                        )
                    hT = wpool.tile([lc, N1], BF16, tag=f"h1T{lt}", name=f"h1T{lt}")
                    nc.vector.tensor_copy(hT[:], ps[0:lc, 0:N1])
                    h1T.append(hT)
                    ps2 = psT.tile([128, 512], BF16, tag="tp", name="tp")
                    for mt, (m0, mc) in enumerate(CH):
                        nc.tensor.transpose(
                            ps2[0:lc, m0 : m0 + mc],
                            wk_b[mt][0:mc, l0 : l0 + lc],
                            ident[0:mc, 0:mc],
                        )
                    wTl = wpool.tile([lc, N1], BF16, tag=f"wkT{lt}", name=f"wkT{lt}")
                    nc.vector.tensor_copy(wTl[:], ps2[0:lc, 0:N1])
                    wkT.append(wTl)

                # W_hkT[m, j] = sum_l wk[m,l] h1[j,l]
                whkT = []
                for mt, (m0, mc) in enumerate(CH):
                    pw = psS.tile([128, 512], F32, tag="ps_st", name="ps_whk")
                    for lt, (l0, lc) in enumerate(CH):
                        nc.tensor.matmul(
                            pw[0:mc, 0:N1],
                            wkT[lt][:, m0 : m0 + mc],
                            h1T[lt][:],
                            start=(lt == 0),
                            stop=(lt == 2),
                        )
                    wTt = wpool.tile(
                        [mc, N1], BF16, tag=f"whkT{mt}", name=f"whkT{mt}"
                    )
                    nc.vector.tensor_copy(wTt[:], pw[0:mc, 0:N1])
                    whkT.append(wTt)

                # per-slice pipeline ---------------------------------------
                xiT = [[None] * 3 for _ in range(P1)]

                def emit_transposes(i):
                    for c in range(2):
                        j0 = c * 128
                        ps = psT.tile([128, 512], BF16, tag="tp", name="tp")
                        for bt in range(4):
                            nc.tensor.transpose(
                                ps[:, bt * 128 : (bt + 1) * 128],
                                deint(xbt[bt][:], i, j0, 128),
                                ident[:],
                            )
                        xi = xtpool.tile(
                            [128, BL], BF16, tag=f"xc{c}", name=f"xiT{i}_{c}"
                        )
                        if c == 0:
                            nc.vector.tensor_copy(xi[:], ps[:])
                        else:
                            nc.scalar.copy(xi[:], ps[:])
                        xiT[i][c] = xi
                    ps = psT.tile([128, 512], BF16, tag="tp", name="tp")
                    for bt in range(4):
                        nc.tensor.transpose(
                            ps[0:32, bt * 128 : (bt + 1) * 128],
                            deint(xbt[bt][:], i, 256, 32),
                            ident[:],
                        )
                    xi = xtpool.tile([32, BL], BF16, tag="xc2", name=f"xiT{i}_2")
                    nc.vector.tensor_copy(xi[:], ps[0:32, :])
                    xiT[i][2] = xi

                def emit_logits(i):
                    for jt, (j0, jc) in enumerate(CH):
                        pst = psS.tile([128, 512], F32, tag="ps_st", name="ps_st")
                        for lt, (l0, lc) in enumerate(CH):
                            nc.tensor.matmul(
                                pst[0:jc, :],
                                whkT[lt][:, j0 : j0 + jc],
                                xiT[i][lt][:],
                                start=(lt == 0),
                                stop=(lt == 2),
                            )
                        col = i * 3 + jt
                        if jt < 2:
                            eout = EA[i][:, jt * BL : (jt + 1) * BL]
                        else:
                            eout = EC[i][:]
                        nc.scalar.activation(
                            eout,
                            pst[0:jc, :],
                            AF.Exp,
                            bias=shiftc[0:jc, 0:1],
                            scale=SCALE,
                            accum_out=densb[0:jc, col : col + 1],
                        )

                def emit_vt(i):
                    for ntc, (n0, ncnt) in enumerate(CH):
                        pv = psV.tile([128, 512], F32, tag="ps_vt", name="ps_vt")
                        for mt, (m0, mc) in enumerate(CH):
                            nc.tensor.matmul(
                                pv[0:ncnt, :],
                                wv_b[mt][:, n0 : n0 + ncnt],
                                xiT[i][mt][:],
                                start=(mt == 0),
                                stop=(mt == 2),
                            )
                        if ntc < 2:
                            zv = zA[i][:, ntc * BL : (ntc + 1) * BL]
                            ev = EA[i][:, ntc * BL : (ntc + 1) * BL]
                        else:
                            zv = zC[i][:]
                            ev = EC[i][:]
                        nc.vector.tensor_mul(zv, pv[0:ncnt, :], ev)

                emit_transposes(0)
                for i in range(P1):
                    if i + 1 < P1:
                        emit_transposes(i + 1)
                    emit_logits(i)
                    if i >= 2 and i <= 5:  # plw casts mid-FIFO (DMAs done by now)
                        emit_plw_cast(2 * (i - 2))
                        emit_plw_cast(2 * (i - 2) + 1)
                    if i == 3:
                        nc.gpsimd.dma_start(cc_in[0][:], densb[:, 0:12])
                        nc.gpsimd.collective_compute(
                            "AllReduce",
                            ALU.add,
                            replica_groups=[list(range(N_CORES))],
                            ins=[cc_in[0][:]],
                            outs=[cc_out[0][:]],
                        )
                nc.gpsimd.dma_start(cc_in[1][:], densb[:, 12:24])
                nc.gpsimd.collective_compute(
                    "AllReduce",
                    ALU.add,
                    replica_groups=[list(range(N_CORES))],
                    ins=[cc_in[1][:]],
                    outs=[cc_out[1][:]],
                )

                # vT + z-muls fill the AllReduce window
                for i in range(P1):
                    emit_vt(i)

                # plw de-interleave (fills the AllReduce window), per seq-half
                for sh in range(2):
                    for i in range(P1):
                        for c in range(2):
                            j0 = c * 128
                            ps = psT2.tile([128, 512], BF16, tag="tp2", name="tp2")
                            for st in range(4):
                                nc.tensor.transpose(
                                    ps[:, st * 128 : (st + 1) * 128],
                                    deint(pwb[sh * 4 + st][:], i, j0, 128),
                                    ident[:],
                                )
                            dst = rk[i][c][:, sh * 512 : (sh + 1) * 512]
                            if (i + c) % 2 == 0:
                                nc.vector.tensor_copy(dst, ps[:])
                            else:
                                nc.scalar.copy(dst, ps[:])
                        ps = psT2.tile([128, 512], BF16, tag="tp2", name="tp2")
                        for st in range(4):
                            nc.tensor.transpose(
                                ps[0:32, st * 128 : (st + 1) * 128],
                                deint(pwb[sh * 4 + st][:], i, 256, 32),
                                ident[:],
                            )
                        nc.vector.tensor_copy(
                            rk2[i][:, sh * 512 : (sh + 1) * 512],
                            ps[0:32, :],
                        )

            # ---------------- AR consume ----------------
            nc.gpsimd.dma_start(den_all[:, 0:12], cc_out[0][:])
            nc.vector.reciprocal(recip[:, 0:12], den_all[:, 0:12])
            nc.gpsimd.dma_start(den_all[:, 12:24], cc_out[1][:])
            nc.vector.reciprocal(recip[:, 12:24], den_all[:, 12:24])
            den_bf = cpool.tile([128, 24], BF16, tag="den_bf", name="den_bf")
            nc.vector.tensor_copy(den_bf[:], den_all[:])

            # ---------------- scan + projection ----------------
            with (
                tc.tile_pool(name="tmp", bufs=1) as tmppool,
                tc.tile_pool(name="osb", bufs=2) as outpool,
                tc.tile_pool(name="psP", bufs=1, space="PSUM") as psP,
            ):
                ttA = tmppool.tile([128, 2 * BL], BF16, tag="ttA", name="ttA")
                tsA = tmppool.tile([128, 2 * BL], BF16, tag="tsA", name="tsA")
                gA = tmppool.tile([128, 2 * BL], BF16, tag="gA", name="gA")
                tt2 = tmppool.tile([32, BL], BF16, tag="tt2", name="tt2")
                ts2 = tmppool.tile([32, BL], BF16, tag="ts2", name="ts2")
                g2 = tmppool.tile([32, BL], BF16, tag="g2", name="g2")

                # re-warm the PE clock right before the projection: junk
                # matmuls anchored on the AllReduce result (overwritten by
                # the start=True proj_len_b pre-load below)
                wps = psP.tile([128, 512], F32, tag="pj00", name="pj_warm")
                for w in range(16):
                    nc.tensor.matmul(
                        wps[0:24, :], den_bf[:], rk[0][0][:, 0:512],
                        start=True, stop=True,
                    )

                # pre-load proj_len_b into the 8 projection PSUM banks
                pps = {}
                for half in range(2):
                    for bc in range(4):
                        pp = psP.tile(
                            [128, 512], F32, tag=f"pj{half}{bc}", name=f"pj{half}{bc}"
                        )
                        nc.tensor.matmul(
                            pp[:],
                            ones_bf[:],
                            plb_sb[0:1, half * 512 : (half + 1) * 512],
                            start=True,
                            stop=False,
                        )
                        pps[(half, bc)] = pp

                def proj_tile(lsrc, off, rsrc, last):
                    for half in range(2):
                        for bc in range(4):
                            nc.tensor.matmul(
                                pps[(half, bc)][:],
                                lsrc[:, off + bc * 128 : off + (bc + 1) * 128],
                                rsrc[:, half * 512 : (half + 1) * 512],
                                start=False,
                                stop=(last and half == 1 and bc == 3),
                            )

                for i in range(P1):
                    cols = [i * 3, i * 3 + 1, i * 3 + 2]
                    zc = zC[i][:]
                    if i == 0:
                        # y_0 = z_0 * recip
                        for c in range(2):
                            nc.scalar.mul(
                                zA[0][:, c * BL : (c + 1) * BL],
                                zA[0][:, c * BL : (c + 1) * BL],
                                mul=recip[0:128, cols[c] : cols[c] + 1],
                            )
                        nc.scalar.mul(
                            zc, zc, mul=recip[0:32, cols[2] : cols[2] + 1]
                        )
                    else:
                        # g = tanh(a1*y + b1) * sigmoid(a2*y + b2)
                        nc.scalar.activation(
                            ttA[:], zA[i - 1][:], AF.Tanh,
                            bias=bcast[0:128, 2:3], scale=bcast[0:128, 0:1],
                        )
                        nc.scalar.activation(
                            tsA[:], zA[i - 1][:], AF.Sigmoid,
                            bias=bcast[0:128, 3:4], scale=bcast[0:128, 1:2],
                        )
                        nc.scalar.activation(
                            tt2[:], zC[i - 1][:], AF.Tanh,
                            bias=bcast[0:32, 2:3], scale=bcast[0:32, 0:1],
                        )
                        nc.scalar.activation(
                            ts2[:], zC[i - 1][:], AF.Sigmoid,
                            bias=bcast[0:32, 3:4], scale=bcast[0:32, 1:2],
                        )
                        nc.vector.tensor_mul(gA[:], ttA[:], tsA[:])
                        nc.gpsimd.tensor_mul(g2[:], tt2[:], ts2[:])
                        # y_i = z_i * recip + g   (normalize folded in)
                        for c in range(2):
                            zv = zA[i][:, c * BL : (c + 1) * BL]
                            nc.vector.scalar_tensor_tensor(
                                zv, zv,
                                recip[0:128, cols[c] : cols[c] + 1],
                                gA[:, c * BL : (c + 1) * BL],
                                op0=ALU.mult, op1=ALU.add,
                            )
                        nc.vector.scalar_tensor_tensor(
                            zc, zc,
                            recip[0:32, cols[2] : cols[2] + 1],
                            g2[:],
                            op0=ALU.mult, op1=ALU.add,
                        )
                    proj_tile(zA[i], 0, rk[i][0], False)
                    proj_tile(zA[i], BL, rk[i][1], False)
                    proj_tile(zC[i], 0, rk2[i], last=(i == P1 - 1))

                for half in range(2):
                    for bc in range(4):
                        ob = outpool.tile([128, 512], F32, tag="osb", name="osb")
                        nc.scalar.copy(ob[:], pps[(half, bc)][:])
                        nc.sync.dma_start(
                            out[bc * 128 : (bc + 1) * 128, half * 512 : (half + 1) * 512],
                            ob[:],
                        )

    nc.compile()
    return nc


_NC = None


def _get_nc():
    global _NC
    if _NC is None:
        _NC = build()
    return _NC


def run(inputs, trace=False):
    nc = _get_nc()
    rep_keys = [
        "w_k1",
        "w_v1",
        "h1",
        "alpha1",
        "alpha2",
        "beta1",
        "beta2",
        "proj_len_w",
        "proj_len_b",
    ]
    x = np.ascontiguousarray(inputs["x"], dtype=np.float32)
    rep = {k: np.ascontiguousarray(inputs[k], dtype=np.float32) for k in rep_keys}
    in_maps = [
        {"x": x[c * BL : (c + 1) * BL], **rep} for c in range(N_CORES)
    ]
    res = run_bass_kernel_spmd(
        nc, in_maps, core_ids=list(range(N_CORES)), trace=trace
    )
    full = np.concatenate([res.results[c]["out"] for c in range(N_CORES)], axis=0)
    return full, res


def kernel(**inputs):
    full, _ = run(inputs, trace=False)
    return full
